# revision 9
# baseline (speedup 1.0000x reference)
"""BatchNorm2d with cubic-spline-interpolated per-channel statistics.

out = x * scale + shift, where scale/shift come from natural-cubic-spline
evaluation of four [T, C] parameter tracks (mean/var/weight/bias) at a
scalar time t:
    scale = weight(t) / sqrt(var(t) + eps)
    shift = bias(t) - mean(t) * scale

Sharding: data-parallel over batch across 8 NeuronCores (4 images each);
the tiny spline parameter tensors are replicated on every core.

Spline evaluation at a fixed scalar t is linear in the knot values, so the
host reduces the time grid to a 10-element basis-weight vector w (by pushing
the identity basis through the spline construction); each core then contracts
the replicated [T, C] parameter tracks with w on-device and streams x through
a fused per-channel affine.
"""

import numpy as np

B, C, H, W = 32, 256, 56, 56
T = 10
EPS = 1e-5
N_CORES = 8
BPC = B // N_CORES        # batch images per core
ROWS = BPC * C            # 1024 rows of [H*W] per core
HWSZ = H * W              # 3136
NBLK = ROWS // 128        # 8 row-blocks of 128 partitions per core

_CACHE = {}


def _spline_basis_weights(times: np.ndarray, t: float) -> np.ndarray:
    """Natural cubic spline eval at t as a linear functional on the knot
    values: eval(times, y, t) == w @ y. Computed by running the spline
    construction on the identity basis (float64 for stability)."""
    times = times.astype(np.float64)
    n = times.shape[0]
    eye = np.eye(n)
    h = np.diff(times)                                   # [n-1]
    slopes = (eye[1:] - eye[:-1]) / h[:, None]           # [n-1, n]
    rhs = 6.0 * (slopes[1:] - slopes[:-1])               # [n-2, n]
    A = (np.diag(2.0 * (h[:-1] + h[1:]))
         + np.diag(h[1:-1], 1)
         + np.diag(h[1:-1], -1))                         # [n-2, n-2]
    m_int = np.linalg.solve(A, rhs)                      # [n-2, n]
    m = np.concatenate([np.zeros((1, n)), m_int, np.zeros((1, n))], axis=0)
    a = eye[:-1]
    b = slopes - h[:, None] * (2.0 * m[:-1] + m[1:]) / 6.0
    c = m[:-1] / 2.0
    d = (m[1:] - m[:-1]) / (6.0 * h[:, None])
    idx = int(np.clip(np.searchsorted(times, t, side="right") - 1, 0, n - 2))
    u = t - times[idx]
    return a[idx] + u * (b[idx] + u * (c[idx] + u * d[idx]))  # [n]


def _build_nc(reps: int = 1):
    # reps>1 re-streams x->y that many times (idempotent); used only by the
    # test harness to measure marginal per-stream HW time.
    import concourse.bacc as bacc
    import concourse.mybir as mybir
    import concourse.tile as tile

    f32 = mybir.dt.float32
    nc = bacc.Bacc("TRN2", target_bir_lowering=False, debug=False)

    x = nc.dram_tensor("x", [ROWS, HWSZ], f32, kind="ExternalInput")
    # pt[c, (p*2+h)*T + k] = param_p[k, h*128 + c]  (p: mean/var/wgt/bias)
    pt = nc.dram_tensor("pt", [128, 8 * T], f32, kind="ExternalInput")
    # wb[c, j*T + k] = w[k]  (spline basis weights, replicated)
    wb = nc.dram_tensor("wb", [128, 8 * T], f32, kind="ExternalInput")
    y = nc.dram_tensor("y", [ROWS, HWSZ], f32, kind="ExternalOutput")

    with tile.TileContext(nc) as tc:
        with (
            tc.tile_pool(name="stats", bufs=1) as sp,
            tc.tile_pool(name="io", bufs=6) as io,
        ):
            # --- per-channel spline stats: contract params with w ---
            # Stats DMAs go on the scalar (ACT) HWDGE ring so the x loads
            # below own the sync ring from the first instruction.
            pt_t = sp.tile([128, 8 * T], f32)
            nc.scalar.dma_start(pt_t[:], pt[:, :])
            wb_t = sp.tile([128, 8 * T], f32)
            nc.scalar.dma_start(wb_t[:], wb[:, :])
            prod = sp.tile([128, 8 * T], f32)
            nc.vector.tensor_mul(prod[:], pt_t[:], wb_t[:])
            # stats cols: mean_lo, mean_hi, var_lo, var_hi, wgt_lo, wgt_hi,
            # bias_lo, bias_hi  (lo/hi = channels 0-127 / 128-255)
            stats = sp.tile([128, 8], f32)
            nc.vector.reduce_sum(
                stats[:],
                prod[:].rearrange("p (j k) -> p j k", k=T),
                axis=mybir.AxisListType.X,
            )
            eps_t = sp.tile([128, 1], f32)
            nc.vector.memset(eps_t[:], EPS)
            std = sp.tile([128, 2], f32)
            nc.scalar.activation(
                std[:], stats[:, 2:4], mybir.ActivationFunctionType.Sqrt,
                bias=eps_t[:],
            )
            inv = sp.tile([128, 2], f32)
            nc.vector.reciprocal(inv[:], std[:])
            scl = sp.tile([128, 2], f32)
            nc.vector.tensor_mul(scl[:], stats[:, 4:6], inv[:])
            tmp = sp.tile([128, 2], f32)
            nc.vector.tensor_mul(tmp[:], stats[:, 0:2], scl[:])
            sh = sp.tile([128, 2], f32)
            nc.vector.tensor_sub(sh[:], stats[:, 6:8], tmp[:])

            # --- stream x through the per-channel affine ---
            # row-block i holds channels (i%2)*128 .. (i%2)*128+127; each
            # block is streamed as eight 196KB chunks. Fine-grained
            # load/store interleave measurably beats large tiles (HW sweep
            # 1.57MB/784/392/196/98KB -> 74/72/68/64/122 us per 25.1MB
            # stream): short alternating bursts let the HBM controller
            # approach single-direction rates, until per-DMA overhead bites
            # at 98KB. Loads ride the sync HWDGE ring, stores the scalar
            # ring, so neither direction queues behind the other.
            hc = HWSZ // 8
            for _ in range(reps):
                for i in range(NBLK):
                    hlf = i % 2
                    for j in range(8):
                        xt = io.tile([128, hc], f32, tag="xt")
                        nc.sync.dma_start(
                            xt[:], x[i * 128:(i + 1) * 128, j * hc:(j + 1) * hc]
                        )
                        yt = io.tile([128, hc], f32, tag="yt")
                        nc.vector.tensor_scalar(
                            yt[:], xt[:],
                            scl[:, hlf:hlf + 1], sh[:, hlf:hlf + 1],
                            op0=mybir.AluOpType.mult, op1=mybir.AluOpType.add,
                        )
                        nc.scalar.dma_start(
                            y[i * 128:(i + 1) * 128, j * hc:(j + 1) * hc], yt[:]
                        )

    nc.compile()
    return nc


def _get_nc():
    if "nc" not in _CACHE:
        _CACHE["nc"] = _build_nc()
    return _CACHE["nc"]


def make_in_maps(x, means, vars_, bnweights, bnbiases, times, t):
    """Shard x by batch; replicate spline params (transposed to a
    channel-partitioned layout) + basis weights on every core."""
    w = _spline_basis_weights(np.asarray(times, np.float32), float(np.asarray(t)[0]))
    params = np.stack(
        [np.asarray(p, np.float32) for p in (means, vars_, bnweights, bnbiases)]
    )                                                     # [4, T, 256]
    p4 = params.reshape(4, T, 2, 128)
    pt = np.ascontiguousarray(
        p4.transpose(3, 0, 2, 1).reshape(128, 8 * T), dtype=np.float32
    )
    wb = np.ascontiguousarray(
        np.broadcast_to(w.astype(np.float32), (128, 8, T)).reshape(128, 8 * T)
    )
    x_np = np.ascontiguousarray(np.asarray(x, np.float32)).reshape(
        N_CORES, ROWS, HWSZ
    )
    return [{"x": x_np[i], "pt": pt, "wb": wb} for i in range(N_CORES)]


def kernel(x, means, vars_, bnweights, bnbiases, times, t):
    from concourse import bass_utils

    nc = _get_nc()
    in_maps = make_in_maps(x, means, vars_, bnweights, bnbiases, times, t)
    res = bass_utils.run_bass_kernel_spmd(nc, in_maps, core_ids=list(range(N_CORES)))
    return np.concatenate(
        [res.results[i]["y"].reshape(BPC, C, H, W) for i in range(N_CORES)], axis=0
    )
